# revision 11
# baseline (speedup 1.0000x reference)
"""ChebConv (K=6) GNN layer on 8 Trainium2 NeuronCores.

Strategy (graph/data parallel):
  - Nodes padded to NPAD and sharded row-wise across 8 cores.
  - Edges sharded by destination core, sorted by (source-chunk, dest), cut
    into 128-edge blocks: each block has one source chunk (int16 gather
    range) and dests within a TSPAN-wide span.
  - Per propagation step, each core:
      * initializes the feature-major working buffer to -Tx_{k-2}
        (Chebyshev recurrence; Tx spills rotate through DRAM),
      * dma_gather's source rows (256B each) from the full node table in HBM
        into edge-major SBUF tiles [128 edges, 64 feat],
      * builds the scatter matrix T on the vector engine (iota/is_equal
        against per-edge dest offsets, scaled by the edge norm which carries
        the 2x of the recurrence),
      * PE matmul per block: slab[64 feat, TSPAN dests] = Y_blk^T @ T_blk
        into a fixed PSUM arena slot,
      * merges each slab into the Tx buffer at its data-dependent dest
        offset with a DVE add through a register-driven dynamic slice
        (dynamic APs on the PE matmul output hang on HW; on DVE they work),
      * accumulates the output term Tx_k @ W_k into a node-major buffer,
      * transposes the new Tx to node-major and AllGathers the shard into
        the next full node table.
  - Host does the O(E) index preprocessing (degree/normalization, sharding,
    block building) and assembles the final output from the 8 shards.
"""

import sys

sys.path.insert(0, "/opt/trn_rl_repo")

import numpy as np

N = 100000
E = 1600000
F = 64
KCHEB = 6
NCORES = 8
NPAD = 100352          # 8 * 12544, and 4 * 25088
SHARD = NPAD // NCORES  # 12544
CH = NPAD // 4          # 25088 rows per int16-addressable gather chunk
NCHUNK = 4
BLK = 128
TSPAN = 48
GBLK = 32              # blocks per gather call
NSLAB = 10             # PSUM slabs per arena bank


# ---------------------------------------------------------------- host prep

def _preprocess(edge_index, edge_weight, n, npad, ncores, ch, tspan):
    shard = npad // ncores
    row = np.asarray(edge_index[0], dtype=np.int64)
    col = np.asarray(edge_index[1], dtype=np.int64)
    w = np.asarray(edge_weight, dtype=np.float32)

    deg = np.zeros(n, np.float32)
    np.add.at(deg, row, w)
    dis = np.where(deg > 0, 1.0 / np.sqrt(np.maximum(deg, 1e-30)), 0.0).astype(
        np.float32
    )
    norm = (-dis[row] * w * dis[col]).astype(np.float32)

    percore = []
    for c in range(ncores):
        base = c * shard
        m = (row >= base) & (row < base + shard)
        drow = (row[m] - base).astype(np.int32)
        ccol = col[m].astype(np.int32)
        cnorm = norm[m]
        cv = ccol // ch
        order = np.lexsort((drow, cv))
        drow, ccol, cnorm, cv = (a[order] for a in (drow, ccol, cnorm, cv))
        ne = len(drow)
        blocks = []  # (start, cnt, chunk, cb)
        i = 0
        while i < ne:
            cvi = int(cv[i])
            cb = min(int(drow[i]), shard - tspan) & ~1
            lim = cb + tspan
            j = i
            while j < ne and j - i < BLK and cv[j] == cvi and drow[j] < lim:
                j += 1
            blocks.append((i, j - i, cvi, cb))
            i = j
        percore.append((blocks, drow, ccol, cnorm))

    # uniform structure: per chunk, max block count over cores
    nbc = [0] * NCHUNK
    for (blocks, _, _, _) in percore:
        cnt = [0] * NCHUNK
        for (_, _, cvi, _) in blocks:
            cnt[cvi] += 1
        for c in range(NCHUNK):
            nbc[c] = max(nbc[c], cnt[c])
    groups = []  # (chunk, nb, bstart)
    bstart = 0
    for cvi in range(NCHUNK):
        if nbc[cvi] == 0:
            continue
        groups.append((cvi, nbc[cvi], bstart))
        bstart += nbc[cvi]
    nbtot = bstart

    gslot = {cvi: bs for (cvi, nb, bs) in groups}

    core_arrays = []
    for (blocks, drow, ccol, cnorm) in percore:
        gidx = np.zeros((nbtot, BLK), np.int16)
        offv = np.full((nbtot, BLK), -1.0, np.float32)
        nrm1 = np.zeros((nbtot, BLK), np.float32)
        cbv = np.zeros(nbtot, np.int32)
        used = [0] * NCHUNK
        for (s, cnt, cvi, cb) in blocks:
            slot = gslot[cvi] + used[cvi]
            used[cvi] += 1
            gidx[slot, :cnt] = (ccol[s : s + cnt] - cvi * ch).astype(np.int16)
            offv[slot, :cnt] = (drow[s : s + cnt] - cb).astype(np.float32)
            nrm1[slot, :cnt] = cnorm[s : s + cnt]
            cbv[slot] = cb
        # wrap gather indices: idx i of a call at [i%16, i//16]; calls are
        # block ranges, so wrap per block ([128] -> [16, 8]) and concat.
        gw = np.ascontiguousarray(
            gidx.reshape(nbtot, 8, 16).transpose(2, 0, 1).reshape(16, nbtot * 8)
        )
        gw = np.tile(gw, (8, 1))  # replicate to 128 partitions for the 8 Q7 cores
        core_arrays.append(
            dict(
                gidx=gw,
                offv=np.ascontiguousarray(offv.T),      # [128, nbtot]
                nrm1=np.ascontiguousarray(nrm1.T),      # [128, nbtot]
                nrm2=np.ascontiguousarray(2.0 * nrm1.T),
                cbv=cbv.reshape(1, nbtot),
            )
        )
    return groups, nbtot, core_arrays


# ---------------------------------------------------------------- device

def _build_program(groups, nbtot, npad, ncores, ch, tspan, ncheb):
    import os
    dbg_nocc = os.environ.get("KDBG_NOCC") == "1"
    dbg_nomerge = os.environ.get("KDBG_NOMERGE") == "1"
    dbg_nogather = os.environ.get("KDBG_NOGATHER") == "1"
    import concourse.bacc as bacc
    import concourse.bass as bass
    import concourse.mybir as mybir
    import concourse.tile as tile
    from concourse.ordered_set import OrderedSet

    f32 = mybir.dt.float32
    shard = npad // ncores
    ntile = shard // 128          # node tiles per shard
    nprop = ncheb - 1

    nc = bacc.Bacc(
        "TRN2", target_bir_lowering=False, debug=False, num_devices=ncores
    )
    xtab = nc.dram_tensor("xtab", [npad, F], f32, kind="ExternalInput")
    xfm = nc.dram_tensor("xfm", [F, shard], f32, kind="ExternalInput")
    gidx_d = nc.dram_tensor("gidx", [128, nbtot * 8], mybir.dt.int16, kind="ExternalInput")
    offv_d = nc.dram_tensor("offv", [128, nbtot], f32, kind="ExternalInput")
    nrm1_d = nc.dram_tensor("nrm1", [128, nbtot], f32, kind="ExternalInput")
    nrm2_d = nc.dram_tensor("nrm2", [128, nbtot], f32, kind="ExternalInput")
    cbv_d = nc.dram_tensor("cbv", [1, nbtot], mybir.dt.int32, kind="ExternalInput")
    wmat_d = nc.dram_tensor("wmat", [F, ncheb * F], f32, kind="ExternalInput")
    brep_d = nc.dram_tensor("brep", [128, F], f32, kind="ExternalInput")
    ident_d = nc.dram_tensor("ident", [F, F], f32, kind="ExternalInput")
    iota_d = nc.dram_tensor("iota", [128, tspan], f32, kind="ExternalInput")
    out_d = nc.dram_tensor("out", [shard, F], f32, kind="ExternalOutput")

    # gather-call ranges: per chunk group, sub-ranges of <= GBLK blocks
    calls = []  # (chunk, bstart, nb)
    for (cvi, nb, bs) in groups:
        for b0 in range(0, nb, GBLK):
            calls.append((cvi, bs + b0, min(GBLK, nb - b0)))

    with tile.TileContext(nc) as tc:
        with (
            tc.tile_pool(name="const", bufs=1) as cp,
            tc.tile_pool(name="tx", bufs=1) as txp,
            tc.tile_pool(name="y", bufs=3) as yp,
            tc.tile_pool(name="t", bufs=3) as tp,
            tc.tile_pool(name="stage", bufs=3) as stp,
            tc.tile_pool(name="psa", bufs=3, space="PSUM") as psap,
            tc.tile_pool(name="pso", bufs=2, space="PSUM") as psop,
            tc.tile_pool(name="pst", bufs=2, space="PSUM") as pstp,
            tc.tile_pool(name="dram", bufs=1, space="DRAM") as dp,
        ):
            # ---- constants / resident tensors
            gidx_sb = cp.tile([128, nbtot * 8], mybir.dt.int16)
            nc.sync.dma_start(gidx_sb[:], gidx_d[:, :])
            offv_sb = cp.tile([128, nbtot], f32)
            nc.sync.dma_start(offv_sb[:], offv_d[:, :])
            nrm1_sb = cp.tile([128, nbtot], f32)
            nc.sync.dma_start(nrm1_sb[:], nrm1_d[:, :])
            nrm2_sb = cp.tile([128, nbtot], f32)
            nc.sync.dma_start(nrm2_sb[:], nrm2_d[:, :])
            cbv_sb = cp.tile([1, nbtot], mybir.dt.int32)
            nc.sync.dma_start(cbv_sb[:], cbv_d[:, :])
            wmat = cp.tile([F, ncheb * F], f32)
            nc.sync.dma_start(wmat[:], wmat_d[:, :])
            brep = cp.tile([128, F], f32)
            nc.sync.dma_start(brep[:], brep_d[:, :])
            ident = cp.tile([F, F], f32)
            nc.sync.dma_start(ident[:], ident_d[:, :])
            iota = cp.tile([128, tspan], f32)
            nc.sync.dma_start(iota[:], iota_d[:, :])

            # Feature-major working buffer; Tx_k spills rotate through DRAM
            bufm = txp.tile([F, shard], f32)
            nc.sync.dma_start(bufm[:, :], xfm[:, :])

            out_nm = txp.tile([128, ntile * F], f32)

            # DRAM: node tables for props 2..nprop, shard bounce buffers
            tables = [
                dp.tile([npad, F], f32, name=f"table{i}", tag=f"table{i}")
                for i in range(nprop - 1)
            ]
            bounces = [
                dp.tile([shard, F], f32, name=f"bounce{i}", tag=f"bounce{i}")
                for i in range(nprop - 1)
            ]
            fmstore = [
                dp.tile([F, shard], f32, name=f"fmstore{i}", tag=f"fmstore{i}")
                for i in range(2)
            ]

            def out_term(k):
                """out_nm (+)= Tx_k @ W_k from bufm, node-major."""
                for t in range(ntile):
                    po = psop.tile([128, F], f32, tag="po")
                    nc.tensor.matmul(
                        po[:],
                        bufm[:, t * 128 : (t + 1) * 128],
                        wmat[:, k * F : (k + 1) * F],
                        start=True, stop=True,
                    )
                    dst = out_nm[:, t * F : (t + 1) * F]
                    if k == 0:
                        nc.scalar.copy(dst, po[:])
                    else:
                        nc.vector.tensor_tensor(dst, po[:], dst, mybir.AluOpType.add)

            out_term(0)
            nc.sync.dma_start(fmstore[0][:, :], bufm[:, :])

            for step in range(1, nprop + 1):
                # table holding Tx_{step-1} (gather source)
                tab = xtab if step == 1 else tables[step - 2]
                txn = bufm
                nrm_sb = nrm1_sb if step == 1 else nrm2_sb

                if step == 1:
                    nc.vector.memset(txn[:, :], 0.0)
                else:
                    # bufm = -Tx_{step-2} from the DRAM spill
                    nc.sync.dma_start(bufm[:, :], fmstore[step % 2][:, :])
                    nc.vector.tensor_scalar_mul(txn[:, :], txn[:, :], -1.0)

                slab = NSLAB
                arena = None
                for (cvi, bs, nb) in calls:
                    y = yp.tile([128, GBLK * F], f32, tag="y")
                    nidx = nb * BLK
                    if dbg_nogather:
                        nc.vector.memset(y[:, : nb * F], 0.0)
                    else:
                        nc.gpsimd.dma_gather(
                            y[:, : nb * F].rearrange("p (n f) -> p n f", f=F),
                            tab[cvi * ch : (cvi + 1) * ch, :],
                            gidx_sb[:, bs * 8 : (bs + nb) * 8],
                            nidx, nidx, F, single_packet=False,
                        )
                    tt = tp.tile([128, GBLK * tspan], f32, tag="t")
                    t3 = tt[:, : nb * tspan].rearrange("p (n j) -> p n j", j=tspan)
                    iota_b = iota[:].rearrange(
                        "p (o j) -> p o j", o=1
                    ).broadcast_to((128, nb, tspan))
                    off_b = offv_sb[:, bs : bs + nb].broadcast_to((128, nb, tspan))
                    nrm_b = nrm_sb[:, bs : bs + nb].broadcast_to((128, nb, tspan))
                    nc.vector.tensor_tensor(
                        t3, iota_b, off_b, mybir.AluOpType.is_equal
                    )
                    nc.vector.tensor_tensor(t3, t3, nrm_b, mybir.AluOpType.mult)
                    _, cbvals = nc.values_load_multi_w_load_instructions(
                        cbv_sb[0:1, bs : bs + nb],
                        engines=OrderedSet([mybir.EngineType.DVE]),
                        min_val=0, max_val=shard - tspan,
                        skip_runtime_bounds_check=True,
                    )
                    for j in range(nb):
                        if slab == NSLAB:
                            arena = psap.tile([F, NSLAB * tspan], f32, tag="arena")
                            slab = 0
                        sl = arena[:, slab * tspan : (slab + 1) * tspan]
                        slab += 1
                        nc.tensor.matmul(
                            sl,
                            y[:, j * F : (j + 1) * F],
                            tt[:, j * tspan : (j + 1) * tspan],
                            start=True, stop=True,
                        )
                        if dbg_nomerge:
                            dyn = bass.ds(0, tspan)
                        else:
                            dyn = bass.ds(cbvals[j], tspan)
                        nc.vector.tensor_tensor(
                            txn[:, dyn], sl, txn[:, dyn], mybir.AluOpType.add
                        )

                out_term(step)
                if step <= nprop - 2:
                    # spill Tx_step (read back at step+2's init)
                    nc.sync.dma_start(fmstore[step % 2][:, :], bufm[:, :])

                if step < nprop:
                    # transpose to node-major and AllGather into next table
                    bounce = bounces[step - 1]
                    src = bufm
                    identh = ident
                    for t0 in range(0, ntile, 8):
                        tn = min(8, ntile - t0)
                        stg = stp.tile([128, 8 * F], f32, tag="stg")
                        for t in range(t0, t0 + tn):
                            pt = pstp.tile([128, F], f32, tag="pt")
                            nc.tensor.transpose(
                                pt[:], src[:, t * 128 : (t + 1) * 128], identh
                            )
                            nc.scalar.copy(
                                stg[:, (t - t0) * F : (t - t0 + 1) * F], pt[:]
                            )
                        nc.sync.dma_start(
                            bounce[t0 * 128 : (t0 + tn) * 128, :].rearrange(
                                "(t p) f -> p t f", p=128
                            ),
                            stg[:, : tn * F].rearrange("p (t f) -> p t f", f=F),
                        )
                    if dbg_nocc:
                        nc.sync.dma_start(
                            tables[step - 1][0 : shard, :], bounce[:, :]
                        )
                    else:
                        nc.gpsimd.collective_compute(
                            "AllGather",
                            mybir.AluOpType.bypass,
                            replica_groups=[list(range(ncores))],
                            ins=[bounce.opt()],
                            outs=[tables[step - 1].opt()],
                        )

            # ---- finalize: bias + relu, store node-major output
            brep_b = brep[:].rearrange("p (o f) -> p o f", o=1).broadcast_to(
                (128, ntile, F)
            )
            o3 = out_nm[:].rearrange("p (t f) -> p t f", f=F)
            nc.vector.tensor_tensor(o3, o3, brep_b, mybir.AluOpType.add)
            nc.scalar.activation(
                out_nm[:], out_nm[:], mybir.ActivationFunctionType.Relu
            )
            nc.sync.dma_start(
                out_d[:, :].rearrange("(t p) f -> p t f", p=128),
                out_nm[:].rearrange("p (t f) -> p t f", f=F),
            )

    nc.compile()
    return nc


# ---------------------------------------------------------------- entry

_CACHE = {}


def _run(x, edge_index, edge_weight, W, b, npad, ncores, ch, tspan,
         trace=False):
    from concourse.bass_utils import run_bass_kernel_spmd

    x = np.asarray(x, dtype=np.float32)
    edge_index = np.asarray(edge_index)
    edge_weight = np.asarray(edge_weight, dtype=np.float32)
    W = np.asarray(W, dtype=np.float32)
    b = np.asarray(b, dtype=np.float32)
    n = x.shape[0]
    ncheb = W.shape[0]
    shard = npad // ncores

    groups, nbtot, core_arrays = _preprocess(
        edge_index, edge_weight, n, npad, ncores, ch, tspan
    )

    key = (tuple(groups), nbtot)
    if key not in _CACHE:
        _CACHE.clear()
        _CACHE[key] = _build_program(
            groups, nbtot, npad, ncores, ch, tspan, ncheb
        )
    nc = _CACHE[key]

    xpad = np.zeros((npad, F), np.float32)
    xpad[:n] = x
    wmat = np.ascontiguousarray(W.transpose(1, 0, 2).reshape(F, ncheb * F))
    brep = np.broadcast_to(b, (128, F)).copy()
    ident = np.eye(F, dtype=np.float32)
    iotav = np.broadcast_to(np.arange(tspan, dtype=np.float32), (128, tspan)).copy()

    in_maps = []
    for c in range(ncores):
        ca = core_arrays[c]
        xfm = np.ascontiguousarray(xpad[c * shard : (c + 1) * shard].T)
        in_maps.append(
            dict(
                xtab=xpad, xfm=xfm,
                gidx=ca["gidx"], offv=ca["offv"], nrm1=ca["nrm1"],
                nrm2=ca["nrm2"], cbv=ca["cbv"],
                wmat=wmat, brep=brep, ident=ident, iota=iotav,
            )
        )
    res = run_bass_kernel_spmd(
        nc, in_maps, core_ids=list(range(ncores)), trace=trace
    )
    out = np.concatenate([res.results[c]["out"] for c in range(ncores)], axis=0)
    return np.ascontiguousarray(out[:n]), res


def kernel(x, edge_index, edge_weight, W, b):
    out, _ = _run(x, edge_index, edge_weight, W, b, NPAD, NCORES, CH, TSPAN)
    return out
